# revision 8
# baseline (speedup 1.0000x reference)
"""Self-attention (SAGAN-style) Trainium2 kernel, v2.

Reference computation (per batch sample):
    theta = w_theta @ x            # [32, 4096]
    phi   = pool2x2(w_phi @ x)     # [32, 1024]
    g     = pool2x2(w_g @ x)       # [128, 1024]
    beta  = softmax(theta.T @ phi, axis=-1)   # [4096, 1024]
    attn  = g @ beta.T             # [128, 4096]
    out   = gamma * (w_o @ attn) + x

Sharding: data-parallel over batch; B=16 over 8 cores -> 2 samples/core.

v2 strategy (vs v1 baseline at ~179us):
  - x is loaded once as fp32 and kept resident in SBUF; projections run as
    fp32r matmuls straight off it (1 cycle/row like bf16), so the casting-DMA
    preload and the fp32 residual re-read are both gone.
  - softmax denominator: instead of a ones-matmul accumulation over all 8
    m-chunks (as costly as the attention itself), the 8 exp tiles are summed
    by an in-place bf16 add chain on DVE and a single K=128 ones-matmul
    broadcasts the per-column sum to all partitions.
  - elementwise work is spread: exps on ScalarE (sole Exp engine), pools /
    copies / residual adds split between Pool(gpsimd) and DVE.
  - PE stream is kept dense by interleaving: score rounds of quarter qt run
    together with attention/epilogue units of quarters < qt and with the next
    sample's projections.
"""

import numpy as np

import concourse.bacc as bacc
import concourse.mybir as mybir
from concourse import tile
from concourse.bass_utils import run_bass_kernel_spmd
from concourse.alu_op_type import AluOpType

F32 = mybir.dt.float32
F32R = mybir.dt.float32r
BF16 = mybir.dt.bfloat16
EXP = mybir.ActivationFunctionType.Exp

B, C, H, W = 16, 256, 64, 64
N = H * W            # 4096
M = N // 4           # 1024
C8 = C // 8          # 32
C2 = C // 2          # 128
NCORES = 8
BPC = B // NCORES    # 2 samples per core
NCH = 512            # n-chunk width for matmul streaming
NNCH = N // NCH      # 8
MC = M // 128        # 8 m-chunks


def build_kernel():
    nc = bacc.Bacc("TRN2", target_bir_lowering=False, debug=False)

    x_d = nc.declare_dram_parameter("x", [BPC, C, N], F32R, isOutput=False)
    wq_d = nc.declare_dram_parameter("wq", [2, 128, 64], F32R, isOutput=False)
    wg_d = nc.declare_dram_parameter("wg", [2, 128, C2], F32R, isOutput=False)
    wo_d = nc.declare_dram_parameter("wo", [2, C2, 128], F32, isOutput=False)
    id_d = nc.declare_dram_parameter("ident", [128, 128], F32, isOutput=False)
    out_d = nc.declare_dram_parameter("out", [BPC, C, N], F32, isOutput=True)

    with tile.TileContext(nc) as tc:
        with (
            tc.tile_pool(name="const", bufs=1) as constp,
            tc.tile_pool(name="xf", bufs=6) as xfp,
            tc.tile_pool(name="proj", bufs=2) as projp,
            tc.tile_pool(name="gts", bufs=1) as gtsp,
            tc.tile_pool(name="exp", bufs=1) as expp,
            tc.tile_pool(name="esum", bufs=2) as esump,
            tc.tile_pool(name="small", bufs=3) as smallp,
            tc.tile_pool(name="outs", bufs=4) as outp,
            tc.tile_pool(name="ps_big", bufs=2, space="PSUM") as psb,
            tc.tile_pool(name="ps_u", bufs=4, space="PSUM") as psu,
        ):
            # ---- weights ----
            wq, wg = [], []
            for cc in range(2):
                t = constp.tile([128, 64], F32R, tag=f"wq{cc}", name=f"wq{cc}")
                nc.sync.dma_start(t[:], wq_d[cc])
                wq.append(t)
                t = constp.tile([128, C2], F32R, tag=f"wg{cc}", name=f"wg{cc}")
                nc.sync.dma_start(t[:], wg_d[cc])
                wg.append(t)
            wo = []
            for oc in range(2):
                t = constp.tile([C2, 128], BF16, tag=f"wo{oc}", name=f"wo{oc}")
                nc.gpsimd.dma_start(t[:], wo_d[oc])  # casting DMA f32->bf16
                wo.append(t)
            id_b = constp.tile([128, 128], BF16, tag="id_b", name="id_b")
            nc.gpsimd.dma_start(id_b[:], id_d[:])
            ones = constp.tile([128, 128], BF16, tag="ones", name="ones")
            nc.gpsimd.memset(ones[:], 1.0)

            # ---- x loads: fp32, kept resident; chunked for early start ----
            # xf[b][(cc, half)] = [128, 2048] fp32
            xf = [dict() for _ in range(BPC)]
            for b in range(BPC):
                for half in range(2):
                    for cc in range(2):
                        t = xfp.tile([128, 2048], F32R, tag="xf",
                                     name=f"xf{b}_{cc}_{half}")
                        xf[b][(cc, half)] = t
            # emission order = priority: b0 pieces first, small leading pieces
            def emit_x_dmas(b, piece_w):
                for half in range(2):
                    for p0 in range(0, 2048, piece_w):
                        for cc in range(2):
                            src = slice(half * 2048 + p0, half * 2048 + p0 + piece_w)
                            nc.sync.dma_start(
                                xf[b][(cc, half)][:, p0:p0 + piece_w],
                                x_d[b, cc * 128:(cc + 1) * 128, src])
            emit_x_dmas(0, 512)
            emit_x_dmas(1, 1024)

            # per-sample state
            st = [dict(ets=[None] * MC, gts=[None] * MC, aps={}, at={},
                       osb={}) for _ in range(BPC)]
            for b in range(BPC):
                for mc in range(MC):
                    st[b]["ets"][mc] = expp.tile(
                        [128, N], BF16, tag=f"ets{mc}", name=f"ets{mc}_{b}")

            pending = []

            def pop(k):
                for _ in range(k):
                    if pending:
                        pending.pop(0)()

            # ---------------- phase emitters ----------------
            def proj_chunk(b, i):
                """Projections for 512-col chunk i of sample b + evacuation."""
                half, off = i // 4, (i % 4) * 512
                s = st[b]
                xs = [xf[b][(cc, half)][:, off:off + 512] for cc in range(2)]
                big = psb.tile([128, 1024], F32, tag="big", name=f"pj{b}_{i}")
                # g-proj into cols 0:512 (full 128 rows)
                for cc in range(2):
                    nc.tensor.matmul(big[:, 0:512], wg[cc][:],
                                     xs[cc],
                                     start=(cc == 0), stop=(cc == 1),
                                     skip_group_check=True)
                # q-proj (theta rows 0:32, phi rows 32:64) into cols 512:1024
                for cc in range(2):
                    nc.tensor.matmul(big[0:64, 512:1024], wq[cc][:],
                                     xs[cc],
                                     start=(cc == 0), stop=(cc == 1),
                                     skip_group_check=True)
                if i == 0:
                    s["th2"] = projp.tile([32, N], BF16, tag="th2", name=f"th2_{b}")
                    s["ph2"] = projp.tile([32, M], BF16, tag="ph2", name=f"ph2_{b}")
                    s["gp"] = projp.tile([C2, M], BF16, tag="gp", name=f"gp_{b}")
                # theta evacuation on ACT (phi is pooled straight from PSUM)
                sl = slice(i * 512, (i + 1) * 512)
                nc.scalar.copy(s["th2"][:, sl], big[0:32, 512:1024])
                # 2x2 maxpools: single DVE reduce over (hb, two) innermost axes,
                # reading PSUM directly
                msl = slice(i * 128, (i + 1) * 128)
                pv = big[32:64, 512:1024].rearrange(
                    "p (h2 hb w2 two) -> p h2 w2 hb two", h2=4, hb=2, w2=32, two=2)
                nc.vector.tensor_reduce(
                    s["ph2"][:, msl].rearrange("p (h2 w2) -> p h2 w2", h2=4, w2=32),
                    pv, mybir.AxisListType.XY, AluOpType.max)
                gv = big[:, 0:512].rearrange(
                    "p (h2 hb w2 two) -> p h2 w2 hb two", h2=4, hb=2, w2=32, two=2)
                nc.vector.tensor_reduce(
                    s["gp"][:, msl].rearrange("p (h2 w2) -> p h2 w2", h2=4, w2=32),
                    gv, mybir.AxisListType.XY, AluOpType.max)

            def tp_chunk(b, mc):
                """Transpose pooled g m-chunk -> gts[mc] (attn stationary)."""
                s = st[b]
                tp = psu.tile([128, 128], BF16, tag="u", name=f"tp{b}_{mc}")
                nc.tensor.transpose(tp[:], s["gp"][:, mc * 128:(mc + 1) * 128],
                                    id_b[:])
                gt = gtsp.tile([128, 128], BF16, tag=f"gt{mc}", name=f"gt{mc}_{b}")
                nc.scalar.copy(gt[:], tp[:])
                s["gts"][mc] = gt

            def score_round(b, qt, mc):
                """Scores + exp for (quarter qt, m-chunk mc) of sample b."""
                s = st[b]
                qsl = slice(qt * 1024, (qt + 1) * 1024)
                sp = psb.tile([128, 1024], F32, tag="big", name=f"sp{b}_{qt}_{mc}")
                for hf in range(2):
                    nsl = slice(qt * 1024 + hf * 512, qt * 1024 + (hf + 1) * 512)
                    nc.tensor.matmul(sp[:, hf * 512:(hf + 1) * 512],
                                     s["ph2"][:, mc * 128:(mc + 1) * 128],
                                     s["th2"][:, nsl], start=True, stop=True)
                nc.scalar.activation(s["ets"][mc][:, qsl], sp[:], EXP)
                # denominator partial-sum chain: DMA accumulate (SBUF->SBUF),
                # triggered from the otherwise-idle GPSIMD queue
                if mc == 0:
                    if qt == 0:
                        s["etsum"] = esump.tile([128, N], BF16, tag="etsum",
                                                name=f"etsum_{b}")
                    nc.gpsimd.dma_start(s["etsum"][:, qsl], s["ets"][0][:, qsl])
                else:
                    nc.gpsimd.dma_start(s["etsum"][:, qsl], s["ets"][mc][:, qsl],
                                        accum_op=AluOpType.add)

            def unit_attn(b, i):
                nsl = slice(i * 512, (i + 1) * 512)
                s = st[b]
                aps = psu.tile([128, 512], F32, tag="u", name=f"aps{b}_{i}")
                s["aps"][i] = aps
                for mc in range(MC):
                    nc.tensor.matmul(aps[:], s["gts"][mc][:],
                                     s["ets"][mc][:, nsl],
                                     start=(mc == 0), stop=(mc == MC - 1),
                                     skip_group_check=True)

            def unit_den(b, i):
                nsl = slice(i * 512, (i + 1) * 512)
                s = st[b]
                dps = psu.tile([128, 512], F32, tag="u", name=f"dps{b}_{i}")
                nc.tensor.matmul(dps[:], ones[:], s["etsum"][:, nsl],
                                 start=True, stop=True)
                rec = smallp.tile([128, 512], F32, tag="rec", name=f"rec{b}_{i}")
                nc.vector.reciprocal_approx_fast(rec[:], dps[:])
                at = smallp.tile([128, 512], BF16, tag="at", name=f"at{b}_{i}")
                nc.vector.scalar_tensor_tensor(
                    at[:], s["aps"][i][:], 1.0, rec[:],
                    AluOpType.bypass, AluOpType.mult)
                s["at"][i] = at

            def unit_out(b, i):
                s = st[b]
                half, off = i // 4, (i % 4) * 512
                pair, lo = i // 2, (i % 2) * 512
                if i % 2 == 0:
                    s["osb"][pair] = [
                        outp.tile([128, 1024], F32, tag="osb",
                                  name=f"osb{b}_{pair}_{oc}") for oc in range(2)]
                for oc in range(2):
                    op = psu.tile([128, 512], F32, tag="u", name=f"op{b}_{i}_{oc}")
                    nc.tensor.matmul(op[:], wo[oc][:], s["at"][i][:],
                                     start=True, stop=True)
                    osb = s["osb"][pair][oc]
                    nc.vector.scalar_tensor_tensor(
                        osb[:, lo:lo + 512], op[:], 1.0,
                        xf[b][(oc, half)][:, off:off + 512].bitcast(F32),
                        AluOpType.bypass, AluOpType.add)
                    if i % 2 == 1:
                        nc.sync.dma_start(
                            out_d[b, oc * 128:(oc + 1) * 128,
                                  pair * 1024:(pair + 1) * 1024], osb[:])

            def queue_units(b):
                # A(i) -> D(i) -> (one A gap) -> O(i); O trails so PE never
                # stalls on the DVE reciprocal latency.
                order = []
                for i in range(NNCH):
                    order.append(("A", i))
                    order.append(("D", i))
                    if i >= 1:
                        order.append(("O", i - 1))
                order.append(("O", NNCH - 1))
                fn = {"A": unit_attn, "D": unit_den, "O": unit_out}
                for kind, i in order:
                    pending.append(lambda k=kind, j=i, bb=b: fn[k](bb, j))

            # ---------------- emission schedule ----------------
            # sample 0: projections + transposes up front
            for i in range(NNCH):
                proj_chunk(0, i)
            for mc in range(MC):
                tp_chunk(0, mc)

            for b in range(BPC):
                queue_units(b)
                nxt = b + 1
                for qt in range(4):
                    for mc in range(MC):
                        score_round(b, qt, mc)
                        if qt >= 1:
                            # interleave next sample's projections (first half
                            # while this sample's residuals still hold xf slots)
                            if nxt < BPC and mc in (1, 5) and qt <= 2:
                                proj_chunk(nxt, (qt - 1) * 2 + (0 if mc == 1 else 1))
                            if mc % 2 == 1:
                                pop(1)
                            if mc in (3, 7):
                                pop(1)
                # drain this sample's units; late ones gate on qt3 exps
                while pending:
                    pop(1)
                if nxt < BPC:
                    for i in range(4, NNCH):
                        proj_chunk(nxt, i)
                    for mc in range(MC):
                        tp_chunk(nxt, mc)

    nc.compile()
    return nc


_NC_CACHE = None


def _get_nc():
    global _NC_CACHE
    if _NC_CACHE is None:
        _NC_CACHE = build_kernel()
    return _NC_CACHE


def prep_inputs(x, w_theta, w_phi, w_g, w_o, gamma):
    """Host-side prep: shard x over 8 cores; transpose/scale/pack weights."""
    x = np.asarray(x, dtype=np.float32).reshape(B, C, N)
    w_theta = np.asarray(w_theta, dtype=np.float32)
    w_phi = np.asarray(w_phi, dtype=np.float32)
    w_g = np.asarray(w_g, dtype=np.float32)
    w_o = np.asarray(w_o, dtype=np.float32)
    gamma = np.float32(gamma)

    # combined projection weight: [theta | phi] along output dim
    wqT = np.concatenate([w_theta.T, w_phi.T], axis=1)       # [256, 64]
    wq = np.ascontiguousarray(wqT.reshape(2, 128, 64))
    wgq = np.ascontiguousarray(w_g.T.reshape(2, 128, C2))
    woT = (gamma * w_o).T                                     # [128, 256]
    wo = np.ascontiguousarray(woT.reshape(C2, 2, 128).transpose(1, 0, 2))
    ident = np.eye(128, dtype=np.float32)

    in_maps = []
    for core in range(NCORES):
        shard = np.ascontiguousarray(x[core * BPC:(core + 1) * BPC])
        in_maps.append({"x": shard, "wq": wq, "wg": wgq, "wo": wo,
                        "ident": ident})
    return in_maps


def run(inputs, trace=False, **kw):
    nc = _get_nc()
    in_maps = prep_inputs(**inputs)
    res = run_bass_kernel_spmd(nc, in_maps, core_ids=list(range(NCORES)),
                               trace=trace, **kw)
    outs = [res.results[i]["out"] for i in range(NCORES)]
    full = np.concatenate(outs, axis=0).reshape(B, C, H, W).astype(np.float32)
    return full, res


def kernel(**inputs):
    full, _ = run(inputs, trace=False)
    return full


# revision 11
# speedup vs baseline: 1.2821x; 1.2821x over previous
"""Self-attention (SAGAN-style) Trainium2 kernel, v2.

Reference computation (per batch sample):
    theta = w_theta @ x            # [32, 4096]
    phi   = pool2x2(w_phi @ x)     # [32, 1024]
    g     = pool2x2(w_g @ x)       # [128, 1024]
    beta  = softmax(theta.T @ phi, axis=-1)   # [4096, 1024]
    attn  = g @ beta.T             # [128, 4096]
    out   = gamma * (w_o @ attn) + x

Sharding: data-parallel over batch; B=16 over 8 cores -> 2 samples/core.

v2 strategy (vs v1 baseline at ~179us):
  - x is loaded once as fp32 and kept resident in SBUF; projections run as
    fp32r matmuls straight off it (1 cycle/row like bf16), so the casting-DMA
    preload and the fp32 residual re-read are both gone.
  - softmax denominator: instead of a ones-matmul accumulation over all 8
    m-chunks (as costly as the attention itself), the 8 exp tiles are summed
    by an in-place bf16 add chain on DVE and a single K=128 ones-matmul
    broadcasts the per-column sum to all partitions.
  - elementwise work is spread: exps on ScalarE (sole Exp engine), pools /
    copies / residual adds split between Pool(gpsimd) and DVE.
  - PE stream is kept dense by interleaving: score rounds of quarter qt run
    together with attention/epilogue units of quarters < qt and with the next
    sample's projections.
"""

import numpy as np

import concourse.bacc as bacc
import concourse.mybir as mybir
from concourse import tile
from concourse.bass_utils import run_bass_kernel_spmd
from concourse.alu_op_type import AluOpType

F32 = mybir.dt.float32
F32R = mybir.dt.float32r
BF16 = mybir.dt.bfloat16
EXP = mybir.ActivationFunctionType.Exp

B, C, H, W = 16, 256, 64, 64
N = H * W            # 4096
M = N // 4           # 1024
C8 = C // 8          # 32
C2 = C // 2          # 128
NCORES = 8
BPC = B // NCORES    # 2 samples per core
NCH = 512            # n-chunk width for matmul streaming
NNCH = N // NCH      # 8
MC = M // 128        # 8 m-chunks


def build_kernel():
    nc = bacc.Bacc("TRN2", target_bir_lowering=False, debug=False)

    x_d = nc.declare_dram_parameter("x", [BPC, C, N], F32R, isOutput=False)
    wq_d = nc.declare_dram_parameter("wq", [2, 128, 64], F32R, isOutput=False)
    wg_d = nc.declare_dram_parameter("wg", [2, 128, C2], F32R, isOutput=False)
    wo_d = nc.declare_dram_parameter("wo", [2, C2, 128], F32, isOutput=False)
    id_d = nc.declare_dram_parameter("ident", [128, 128], F32, isOutput=False)
    idr_d = nc.declare_dram_parameter("identr", [128, 128], F32R, isOutput=False)
    out_d = nc.declare_dram_parameter("out", [BPC, C, N], F32, isOutput=True)

    with tile.TileContext(nc) as tc:
        with (
            tc.tile_pool(name="const", bufs=1) as constp,
            tc.tile_pool(name="xf", bufs=6) as xfp,
            tc.tile_pool(name="proj", bufs=2) as projp,
            tc.tile_pool(name="gts", bufs=1) as gtsp,
            tc.tile_pool(name="exp", bufs=1) as expp,
            tc.tile_pool(name="esum", bufs=1) as esump,
            tc.tile_pool(name="small", bufs=3) as smallp,
            tc.tile_pool(name="outs", bufs=4) as outp,
            tc.tile_pool(name="ps_big", bufs=2, space="PSUM") as psb,
            tc.tile_pool(name="ps_u", bufs=4, space="PSUM") as psu,
        ):
            # ---- weights ----
            wq, wg = [], []
            for cc in range(2):
                t = constp.tile([128, 64], F32R, tag=f"wq{cc}", name=f"wq{cc}")
                nc.sync.dma_start(t[:], wq_d[cc])
                wq.append(t)
                t = constp.tile([128, C2], F32R, tag=f"wg{cc}", name=f"wg{cc}")
                nc.sync.dma_start(t[:], wg_d[cc])
                wg.append(t)
            wo = []
            for oc in range(2):
                t = constp.tile([C2, 128], BF16, tag=f"wo{oc}", name=f"wo{oc}")
                nc.gpsimd.dma_start(t[:], wo_d[oc])  # casting DMA f32->bf16
                wo.append(t)
            id_b = constp.tile([128, 128], BF16, tag="id_b", name="id_b")
            nc.gpsimd.dma_start(id_b[:], id_d[:])
            id_r = constp.tile([128, 128], F32R, tag="id_r", name="id_r")
            nc.sync.dma_start(id_r[:], idr_d[:])
            ones = constp.tile([128, 128], BF16, tag="ones", name="ones")
            nc.gpsimd.memset(ones[:], 1.0)

            # ---- x loads: fp32, kept resident; chunked for early start ----
            # xf[b][(cc, half)] = [128, 2048] fp32
            xf = [dict() for _ in range(BPC)]
            for b in range(BPC):
                for half in range(2):
                    for cc in range(2):
                        t = xfp.tile([128, 2048], F32R, tag="xf",
                                     name=f"xf{b}_{cc}_{half}")
                        xf[b][(cc, half)] = t
            # emission order = priority: b0 pieces first, small leading pieces
            def emit_x_dmas(b, piece_w):
                for half in range(2):
                    for p0 in range(0, 2048, piece_w):
                        for cc in range(2):
                            src = slice(half * 2048 + p0, half * 2048 + p0 + piece_w)
                            nc.sync.dma_start(
                                xf[b][(cc, half)][:, p0:p0 + piece_w],
                                x_d[b, cc * 128:(cc + 1) * 128, src])
            emit_x_dmas(0, 512)
            emit_x_dmas(1, 1024)

            # per-sample state
            st = [dict(ets=[None] * MC, gts=[None] * MC, aps={}, at={},
                       osb={}) for _ in range(BPC)]
            for b in range(BPC):
                for mc in range(MC):
                    st[b]["ets"][mc] = expp.tile(
                        [128, N], BF16, tag=f"ets{mc}", name=f"ets{mc}_{b}")

            pending = []

            def pop(k):
                for _ in range(k):
                    if pending:
                        pending.pop(0)()

            # ---------------- phase emitters ----------------
            def proj_chunk(b, i):
                """Projections for 512-col chunk i of sample b + evacuation."""
                half, off = i // 4, (i % 4) * 512
                s = st[b]
                xs = [xf[b][(cc, half)][:, off:off + 512] for cc in range(2)]
                big = psb.tile([128, 1024], F32, tag="big", name=f"pj{b}_{i}")
                # g-proj into cols 0:512 (full 128 rows)
                for cc in range(2):
                    nc.tensor.matmul(big[:, 0:512], wg[cc][:],
                                     xs[cc],
                                     start=(cc == 0), stop=(cc == 1),
                                     skip_group_check=True)
                # q-proj (theta rows 0:32, phi rows 32:64) into cols 512:1024
                for cc in range(2):
                    nc.tensor.matmul(big[0:64, 512:1024], wq[cc][:],
                                     xs[cc],
                                     start=(cc == 0), stop=(cc == 1),
                                     skip_group_check=True)
                if i == 0:
                    s["th2"] = projp.tile([32, N], BF16, tag="th2", name=f"th2_{b}")
                    s["ph2"] = projp.tile([32, M], BF16, tag="ph2", name=f"ph2_{b}")
                    s["gp"] = projp.tile([C2, M], BF16, tag="gp", name=f"gp_{b}")
                # theta evacuation on ACT (phi is pooled straight from PSUM)
                sl = slice(i * 512, (i + 1) * 512)
                nc.scalar.copy(s["th2"][:, sl], big[0:32, 512:1024])
                # 2x2 maxpools: single DVE reduce over (hb, two) innermost axes,
                # reading PSUM directly
                msl = slice(i * 128, (i + 1) * 128)
                pv = big[32:64, 512:1024].rearrange(
                    "p (h2 hb w2 two) -> p h2 w2 hb two", h2=4, hb=2, w2=32, two=2)
                nc.vector.tensor_reduce(
                    s["ph2"][:, msl].rearrange("p (h2 w2) -> p h2 w2", h2=4, w2=32),
                    pv, mybir.AxisListType.XY, AluOpType.max)
                gv = big[:, 0:512].rearrange(
                    "p (h2 hb w2 two) -> p h2 w2 hb two", h2=4, hb=2, w2=32, two=2)
                nc.vector.tensor_reduce(
                    s["gp"][:, msl].rearrange("p (h2 w2) -> p h2 w2", h2=4, w2=32),
                    gv, mybir.AxisListType.XY, AluOpType.max)

            def tp_chunk(b, mc):
                """Transpose pooled g m-chunk -> gts[mc] (attn stationary)."""
                s = st[b]
                tp = psu.tile([128, 128], BF16, tag="u", name=f"tp{b}_{mc}")
                nc.tensor.transpose(tp[:], s["gp"][:, mc * 128:(mc + 1) * 128],
                                    id_b[:])
                gt = gtsp.tile([128, 128], BF16, tag=f"gt{mc}", name=f"gt{mc}_{b}")
                nc.vector.tensor_copy(gt[:], tp[:])
                s["gts"][mc] = gt

            def score_round(b, qt, mc):
                """Scores + exp for (quarter qt, m-chunk mc) of sample b."""
                s = st[b]
                qsl = slice(qt * 1024, (qt + 1) * 1024)
                sp = psb.tile([128, 1024], F32, tag="big", name=f"sp{b}_{qt}_{mc}")
                for hf in range(2):
                    nsl = slice(qt * 1024 + hf * 512, qt * 1024 + (hf + 1) * 512)
                    nc.tensor.matmul(sp[:, hf * 512:(hf + 1) * 512],
                                     s["ph2"][:, mc * 128:(mc + 1) * 128],
                                     s["th2"][:, nsl], start=True, stop=True)
                nc.scalar.activation(s["ets"][mc][:, qsl], sp[:], EXP)
                # denominator partials: independent pairwise adds on DVE;
                # the den matmul accumulates over the 4 partial tiles
                if mc % 2 == 1:
                    p = mc // 2
                    if qt == 0 and mc == 1:
                        s["psum4"] = [
                            esump.tile([128, N], BF16, tag=f"psum{j}",
                                       name=f"psum{j}_{b}") for j in range(4)]
                    nc.vector.tensor_tensor(s["psum4"][p][:, qsl],
                                            s["ets"][mc - 1][:, qsl],
                                            s["ets"][mc][:, qsl], AluOpType.add)

            def unit_attn(b, i):
                nsl = slice(i * 512, (i + 1) * 512)
                s = st[b]
                aps = psu.tile([128, 512], F32, tag="u", name=f"aps{b}_{i}")
                s["aps"][i] = aps
                for mc in range(MC):
                    nc.tensor.matmul(aps[:], s["gts"][mc][:],
                                     s["ets"][mc][:, nsl],
                                     start=(mc == 0), stop=(mc == MC - 1),
                                     skip_group_check=True)

            def unit_den(b, i):
                nsl = slice(i * 512, (i + 1) * 512)
                s = st[b]
                dps = psu.tile([128, 512], F32, tag="u", name=f"dps{b}_{i}")
                for p in range(4):
                    nc.tensor.matmul(dps[:], ones[:], s["psum4"][p][:, nsl],
                                     start=(p == 0), stop=(p == 3),
                                     skip_group_check=True)

                rec = smallp.tile([128, 512], F32, tag="rec", name=f"rec{b}_{i}")
                nc.vector.reciprocal_approx_fast(rec[:], dps[:])
                at = smallp.tile([128, 512], BF16, tag="at", name=f"at{b}_{i}")
                nc.vector.scalar_tensor_tensor(
                    at[:], s["aps"][i][:], 1.0, rec[:],
                    AluOpType.bypass, AluOpType.mult)
                s["at"][i] = at

            def unit_out(b, i):
                s = st[b]
                nsl = slice(i * 512, (i + 1) * 512)
                half, off = i // 4, (i % 4) * 512
                for oc in range(2):
                    op = psu.tile([128, 512], F32, tag="u", name=f"op{b}_{i}_{oc}")
                    # residual: identity matmul copies x into PSUM, then the
                    # out-projection accumulates gamma*w_o@attn on top
                    nc.tensor.matmul(op[:], id_r[:],
                                     xf[b][(oc, half)][:, off:off + 512],
                                     start=True, stop=False,
                                     skip_group_check=True)
                    nc.tensor.matmul(op[:], wo[oc][:], s["at"][i][:],
                                     start=False, stop=True,
                                     skip_group_check=True)
                    osb = outp.tile([128, 512], F32, tag="osb",
                                    name=f"osb{b}_{i}_{oc}")
                    nc.scalar.copy(osb[:], op[:])
                    nc.sync.dma_start(out_d[b, oc * 128:(oc + 1) * 128, nsl],
                                      osb[:])

            def queue_units(b):
                # A(i) -> D(i) -> (one A gap) -> O(i); O trails so PE never
                # stalls on the DVE reciprocal latency.
                order = []
                for i in range(NNCH):
                    order.append(("A", i))
                    order.append(("D", i))
                    if i >= 1:
                        order.append(("O", i - 1))
                order.append(("O", NNCH - 1))
                fn = {"A": unit_attn, "D": unit_den, "O": unit_out}
                for kind, i in order:
                    pending.append(lambda k=kind, j=i, bb=b: fn[k](bb, j))

            # ---------------- emission schedule ----------------
            # sample 0: projections + transposes up front
            for i in range(NNCH):
                proj_chunk(0, i)
            for mc in range(MC):
                tp_chunk(0, mc)

            for b in range(BPC):
                queue_units(b)
                nxt = b + 1
                for qt in range(4):
                    for mc in range(MC):
                        score_round(b, qt, mc)
                        if qt >= 1:
                            # interleave next sample's projections (first half
                            # while this sample's residuals still hold xf slots)
                            if nxt < BPC and mc in (1, 5) and qt <= 2:
                                proj_chunk(nxt, (qt - 1) * 2 + (0 if mc == 1 else 1))
                            if mc % 2 == 1:
                                pop(1)
                            if mc in (3, 7):
                                pop(1)
                # drain this sample's units; late ones gate on qt3 exps
                while pending:
                    pop(1)
                if nxt < BPC:
                    for i in range(4, NNCH):
                        proj_chunk(nxt, i)
                    for mc in range(MC):
                        tp_chunk(nxt, mc)

    nc.compile()
    return nc


_NC_CACHE = None


def _get_nc():
    global _NC_CACHE
    if _NC_CACHE is None:
        _NC_CACHE = build_kernel()
    return _NC_CACHE


def prep_inputs(x, w_theta, w_phi, w_g, w_o, gamma):
    """Host-side prep: shard x over 8 cores; transpose/scale/pack weights."""
    x = np.asarray(x, dtype=np.float32).reshape(B, C, N)
    w_theta = np.asarray(w_theta, dtype=np.float32)
    w_phi = np.asarray(w_phi, dtype=np.float32)
    w_g = np.asarray(w_g, dtype=np.float32)
    w_o = np.asarray(w_o, dtype=np.float32)
    gamma = np.float32(gamma)

    # combined projection weight: [theta | phi] along output dim
    wqT = np.concatenate([w_theta.T, w_phi.T], axis=1)       # [256, 64]
    wq = np.ascontiguousarray(wqT.reshape(2, 128, 64))
    wgq = np.ascontiguousarray(w_g.T.reshape(2, 128, C2))
    woT = (gamma * w_o).T                                     # [128, 256]
    wo = np.ascontiguousarray(woT.reshape(C2, 2, 128).transpose(1, 0, 2))
    ident = np.eye(128, dtype=np.float32)

    in_maps = []
    for core in range(NCORES):
        shard = np.ascontiguousarray(x[core * BPC:(core + 1) * BPC])
        in_maps.append({"x": shard, "wq": wq, "wg": wgq, "wo": wo,
                        "ident": ident, "identr": ident})
    return in_maps


def run(inputs, trace=False, **kw):
    nc = _get_nc()
    in_maps = prep_inputs(**inputs)
    res = run_bass_kernel_spmd(nc, in_maps, core_ids=list(range(NCORES)),
                               trace=trace, **kw)
    outs = [res.results[i]["out"] for i in range(NCORES)]
    full = np.concatenate(outs, axis=0).reshape(B, C, H, W).astype(np.float32)
    return full, res


def kernel(**inputs):
    full, _ = run(inputs, trace=False)
    return full


# revision 12
# speedup vs baseline: 1.2926x; 1.0082x over previous
"""Self-attention (SAGAN-style) Trainium2 kernel, v2.

Reference computation (per batch sample):
    theta = w_theta @ x            # [32, 4096]
    phi   = pool2x2(w_phi @ x)     # [32, 1024]
    g     = pool2x2(w_g @ x)       # [128, 1024]
    beta  = softmax(theta.T @ phi, axis=-1)   # [4096, 1024]
    attn  = g @ beta.T             # [128, 4096]
    out   = gamma * (w_o @ attn) + x

Sharding: data-parallel over batch; B=16 over 8 cores -> 2 samples/core.

v2 strategy (vs v1 baseline at ~179us):
  - x is loaded once as fp32 and kept resident in SBUF; projections run as
    fp32r matmuls straight off it (1 cycle/row like bf16), so the casting-DMA
    preload and the fp32 residual re-read are both gone.
  - softmax denominator: instead of a ones-matmul accumulation over all 8
    m-chunks (as costly as the attention itself), the 8 exp tiles are summed
    by an in-place bf16 add chain on DVE and a single K=128 ones-matmul
    broadcasts the per-column sum to all partitions.
  - elementwise work is spread: exps on ScalarE (sole Exp engine), pools /
    copies / residual adds split between Pool(gpsimd) and DVE.
  - PE stream is kept dense by interleaving: score rounds of quarter qt run
    together with attention/epilogue units of quarters < qt and with the next
    sample's projections.
"""

import numpy as np

import concourse.bacc as bacc
import concourse.mybir as mybir
from concourse import tile
from concourse.bass_utils import run_bass_kernel_spmd
from concourse.alu_op_type import AluOpType

F32 = mybir.dt.float32
F32R = mybir.dt.float32r
BF16 = mybir.dt.bfloat16
EXP = mybir.ActivationFunctionType.Exp

B, C, H, W = 16, 256, 64, 64
N = H * W            # 4096
M = N // 4           # 1024
C8 = C // 8          # 32
C2 = C // 2          # 128
NCORES = 8
BPC = B // NCORES    # 2 samples per core
NCH = 512            # n-chunk width for matmul streaming
NNCH = N // NCH      # 8
MC = M // 128        # 8 m-chunks


def build_kernel():
    nc = bacc.Bacc("TRN2", target_bir_lowering=False, debug=False)

    x_d = nc.declare_dram_parameter("x", [BPC, C, N], F32R, isOutput=False)
    wq_d = nc.declare_dram_parameter("wq", [2, 128, 64], F32R, isOutput=False)
    wg_d = nc.declare_dram_parameter("wg", [2, 128, C2], F32R, isOutput=False)
    wo_d = nc.declare_dram_parameter("wo", [2, C2, 128], F32, isOutput=False)
    id_d = nc.declare_dram_parameter("ident", [128, 128], F32, isOutput=False)
    idr_d = nc.declare_dram_parameter("identr", [128, 128], F32R, isOutput=False)
    out_d = nc.declare_dram_parameter("out", [BPC, C, N], F32, isOutput=True)

    with tile.TileContext(nc) as tc:
        with (
            tc.tile_pool(name="const", bufs=1) as constp,
            tc.tile_pool(name="xf", bufs=6) as xfp,
            tc.tile_pool(name="proj", bufs=2) as projp,
            tc.tile_pool(name="gts", bufs=1) as gtsp,
            tc.tile_pool(name="exp", bufs=1) as expp,
            tc.tile_pool(name="esum", bufs=1) as esump,
            tc.tile_pool(name="small", bufs=3) as smallp,
            tc.tile_pool(name="outs", bufs=4) as outp,
            tc.tile_pool(name="ps_big", bufs=2, space="PSUM") as psb,
            tc.tile_pool(name="ps_u", bufs=4, space="PSUM") as psu,
        ):
            # ---- weights ----
            wq, wg = [], []
            for cc in range(2):
                t = constp.tile([128, 64], F32R, tag=f"wq{cc}", name=f"wq{cc}")
                nc.scalar.dma_start(t[:], wq_d[cc])
                wq.append(t)
                t = constp.tile([128, C2], F32R, tag=f"wg{cc}", name=f"wg{cc}")
                nc.scalar.dma_start(t[:], wg_d[cc])
                wg.append(t)
            wo = []
            for oc in range(2):
                t = constp.tile([C2, 128], BF16, tag=f"wo{oc}", name=f"wo{oc}")
                nc.gpsimd.dma_start(t[:], wo_d[oc])  # casting DMA f32->bf16
                wo.append(t)
            id_b = constp.tile([128, 128], BF16, tag="id_b", name="id_b")
            nc.gpsimd.dma_start(id_b[:], id_d[:])
            id_r = constp.tile([128, 128], F32R, tag="id_r", name="id_r")
            nc.scalar.dma_start(id_r[:], idr_d[:])
            ones = constp.tile([128, 128], BF16, tag="ones", name="ones")
            nc.gpsimd.memset(ones[:], 1.0)

            # ---- x loads: fp32, kept resident; chunked for early start ----
            # xf[b][(cc, half)] = [128, 2048] fp32
            xf = [dict() for _ in range(BPC)]
            for b in range(BPC):
                for half in range(2):
                    for cc in range(2):
                        t = xfp.tile([128, 2048], F32R, tag="xf",
                                     name=f"xf{b}_{cc}_{half}")
                        xf[b][(cc, half)] = t
            # emission order = priority: b0 pieces first, small leading pieces
            def emit_x_dmas(b, piece_w):
                for half in range(2):
                    for p0 in range(0, 2048, piece_w):
                        for cc in range(2):
                            src = slice(half * 2048 + p0, half * 2048 + p0 + piece_w)
                            nc.sync.dma_start(
                                xf[b][(cc, half)][:, p0:p0 + piece_w],
                                x_d[b, cc * 128:(cc + 1) * 128, src])
            emit_x_dmas(0, 512)
            emit_x_dmas(1, 1024)

            # per-sample state
            st = [dict(ets=[None] * MC, gts=[None] * MC, aps={}, at={},
                       osb={}) for _ in range(BPC)]
            for b in range(BPC):
                for mc in range(MC):
                    st[b]["ets"][mc] = expp.tile(
                        [128, N], BF16, tag=f"ets{mc}", name=f"ets{mc}_{b}")

            pending = []

            def pop(k):
                for _ in range(k):
                    if pending:
                        pending.pop(0)()

            # ---------------- phase emitters ----------------
            def proj_chunk(b, i):
                """Projections for 512-col chunk i of sample b + evacuation."""
                half, off = i // 4, (i % 4) * 512
                s = st[b]
                xs = [xf[b][(cc, half)][:, off:off + 512] for cc in range(2)]
                big = psb.tile([128, 1024], F32, tag="big", name=f"pj{b}_{i}")
                # g-proj into cols 0:512 (full 128 rows)
                for cc in range(2):
                    nc.tensor.matmul(big[:, 0:512], wg[cc][:],
                                     xs[cc],
                                     start=(cc == 0), stop=(cc == 1),
                                     skip_group_check=True)
                # q-proj (theta rows 0:32, phi rows 32:64) into cols 512:1024
                for cc in range(2):
                    nc.tensor.matmul(big[0:64, 512:1024], wq[cc][:],
                                     xs[cc],
                                     start=(cc == 0), stop=(cc == 1),
                                     skip_group_check=True)
                if i == 0:
                    s["th2"] = projp.tile([32, N], BF16, tag="th2", name=f"th2_{b}")
                    s["ph2"] = projp.tile([32, M], BF16, tag="ph2", name=f"ph2_{b}")
                    s["gp"] = projp.tile([C2, M], BF16, tag="gp", name=f"gp_{b}")
                # theta evacuation on ACT (phi is pooled straight from PSUM)
                sl = slice(i * 512, (i + 1) * 512)
                nc.scalar.copy(s["th2"][:, sl], big[0:32, 512:1024])
                # 2x2 maxpools: single DVE reduce over (hb, two) innermost axes,
                # reading PSUM directly
                msl = slice(i * 128, (i + 1) * 128)
                pv = big[32:64, 512:1024].rearrange(
                    "p (h2 hb w2 two) -> p h2 w2 hb two", h2=4, hb=2, w2=32, two=2)
                nc.vector.tensor_reduce(
                    s["ph2"][:, msl].rearrange("p (h2 w2) -> p h2 w2", h2=4, w2=32),
                    pv, mybir.AxisListType.XY, AluOpType.max)
                gv = big[:, 0:512].rearrange(
                    "p (h2 hb w2 two) -> p h2 w2 hb two", h2=4, hb=2, w2=32, two=2)
                nc.vector.tensor_reduce(
                    s["gp"][:, msl].rearrange("p (h2 w2) -> p h2 w2", h2=4, w2=32),
                    gv, mybir.AxisListType.XY, AluOpType.max)

            def tp_chunk(b, mc):
                """Transpose pooled g m-chunk -> gts[mc] (attn stationary)."""
                s = st[b]
                tp = psu.tile([128, 128], BF16, tag="u", name=f"tp{b}_{mc}")
                nc.tensor.transpose(tp[:], s["gp"][:, mc * 128:(mc + 1) * 128],
                                    id_b[:])
                gt = gtsp.tile([128, 128], BF16, tag=f"gt{mc}", name=f"gt{mc}_{b}")
                nc.vector.tensor_copy(gt[:], tp[:])
                s["gts"][mc] = gt

            def score_round(b, qt, mc):
                """Scores + exp for (quarter qt, m-chunk mc) of sample b."""
                s = st[b]
                qsl = slice(qt * 1024, (qt + 1) * 1024)
                sp = psb.tile([128, 1024], F32, tag="big", name=f"sp{b}_{qt}_{mc}")
                for hf in range(2):
                    nsl = slice(qt * 1024 + hf * 512, qt * 1024 + (hf + 1) * 512)
                    nc.tensor.matmul(sp[:, hf * 512:(hf + 1) * 512],
                                     s["ph2"][:, mc * 128:(mc + 1) * 128],
                                     s["th2"][:, nsl], start=True, stop=True)
                nc.scalar.activation(s["ets"][mc][:, qsl], sp[:], EXP)
                # denominator partials on DVE: pairwise adds + in-place tree
                # combine; the den matmul then needs a single K=128 pass
                if mc % 2 == 1:
                    p = mc // 2
                    if qt == 0 and mc == 1:
                        s["psum4"] = [
                            esump.tile([128, N], BF16, tag=f"psum{j}",
                                       name=f"psum{j}_{b}") for j in range(4)]
                    ps4 = s["psum4"]
                    nc.vector.tensor_tensor(ps4[p][:, qsl],
                                            s["ets"][mc - 1][:, qsl],
                                            s["ets"][mc][:, qsl], AluOpType.add)
                    if mc == 7:
                        nc.vector.tensor_tensor(ps4[0][:, qsl], ps4[0][:, qsl],
                                                ps4[1][:, qsl], AluOpType.add)
                        nc.vector.tensor_tensor(ps4[2][:, qsl], ps4[2][:, qsl],
                                                ps4[3][:, qsl], AluOpType.add)
                        nc.vector.tensor_tensor(ps4[0][:, qsl], ps4[0][:, qsl],
                                                ps4[2][:, qsl], AluOpType.add)

            def unit_attn(b, i):
                nsl = slice(i * 512, (i + 1) * 512)
                s = st[b]
                aps = psu.tile([128, 512], F32, tag="u", name=f"aps{b}_{i}")
                s["aps"][i] = aps
                for mc in range(MC):
                    nc.tensor.matmul(aps[:], s["gts"][mc][:],
                                     s["ets"][mc][:, nsl],
                                     start=(mc == 0), stop=(mc == MC - 1),
                                     skip_group_check=True)

            def unit_den(b, i):
                nsl = slice(i * 512, (i + 1) * 512)
                s = st[b]
                dps = psu.tile([128, 512], F32, tag="u", name=f"dps{b}_{i}")
                nc.tensor.matmul(dps[:], ones[:], s["psum4"][0][:, nsl],
                                 start=True, stop=True)

                rec = smallp.tile([128, 512], F32, tag="rec", name=f"rec{b}_{i}")
                nc.vector.reciprocal_approx_fast(rec[:], dps[:])
                at = smallp.tile([128, 512], BF16, tag="at", name=f"at{b}_{i}")
                nc.vector.scalar_tensor_tensor(
                    at[:], s["aps"][i][:], 1.0, rec[:],
                    AluOpType.bypass, AluOpType.mult)
                s["at"][i] = at

            def unit_out(b, i):
                s = st[b]
                nsl = slice(i * 512, (i + 1) * 512)
                half, off = i // 4, (i % 4) * 512
                for oc in range(2):
                    op = psu.tile([128, 512], F32, tag="u", name=f"op{b}_{i}_{oc}")
                    # residual: identity matmul copies x into PSUM, then the
                    # out-projection accumulates gamma*w_o@attn on top
                    nc.tensor.matmul(op[:], id_r[:],
                                     xf[b][(oc, half)][:, off:off + 512],
                                     start=True, stop=False,
                                     skip_group_check=True)
                    nc.tensor.matmul(op[:], wo[oc][:], s["at"][i][:],
                                     start=False, stop=True,
                                     skip_group_check=True)
                    osb = outp.tile([128, 512], F32, tag="osb",
                                    name=f"osb{b}_{i}_{oc}")
                    nc.scalar.copy(osb[:], op[:])
                    nc.sync.dma_start(out_d[b, oc * 128:(oc + 1) * 128, nsl],
                                      osb[:])

            def queue_units(b):
                # A(i) -> D(i) -> (one A gap) -> O(i); O trails so PE never
                # stalls on the DVE reciprocal latency.
                order = []
                for i in range(NNCH):
                    order.append(("A", i))
                    order.append(("D", i))
                    if i >= 1:
                        order.append(("O", i - 1))
                order.append(("O", NNCH - 1))
                fn = {"A": unit_attn, "D": unit_den, "O": unit_out}
                for kind, i in order:
                    pending.append(lambda k=kind, j=i, bb=b: fn[k](bb, j))

            # ---------------- emission schedule ----------------
            # sample 0: projections + transposes up front
            for i in range(NNCH):
                proj_chunk(0, i)
            for mc in range(MC):
                tp_chunk(0, mc)

            for b in range(BPC):
                queue_units(b)
                nxt = b + 1
                for qt in range(4):
                    for mc in range(MC):
                        score_round(b, qt, mc)
                        if qt >= 1:
                            # interleave next sample's projections (first half
                            # while this sample's residuals still hold xf slots)
                            if nxt < BPC and mc in (1, 5) and qt <= 2:
                                proj_chunk(nxt, (qt - 1) * 2 + (0 if mc == 1 else 1))
                            if mc % 2 == 1:
                                pop(1)
                            if mc in (3, 7):
                                pop(1)
                # drain this sample's units; late ones gate on qt3 exps
                while pending:
                    pop(1)
                if nxt < BPC:
                    for i in range(4, NNCH):
                        proj_chunk(nxt, i)
                    for mc in range(MC):
                        tp_chunk(nxt, mc)

    nc.compile()
    return nc


_NC_CACHE = None


def _get_nc():
    global _NC_CACHE
    if _NC_CACHE is None:
        _NC_CACHE = build_kernel()
    return _NC_CACHE


def prep_inputs(x, w_theta, w_phi, w_g, w_o, gamma):
    """Host-side prep: shard x over 8 cores; transpose/scale/pack weights."""
    x = np.asarray(x, dtype=np.float32).reshape(B, C, N)
    w_theta = np.asarray(w_theta, dtype=np.float32)
    w_phi = np.asarray(w_phi, dtype=np.float32)
    w_g = np.asarray(w_g, dtype=np.float32)
    w_o = np.asarray(w_o, dtype=np.float32)
    gamma = np.float32(gamma)

    # combined projection weight: [theta | phi] along output dim
    wqT = np.concatenate([w_theta.T, w_phi.T], axis=1)       # [256, 64]
    wq = np.ascontiguousarray(wqT.reshape(2, 128, 64))
    wgq = np.ascontiguousarray(w_g.T.reshape(2, 128, C2))
    woT = (gamma * w_o).T                                     # [128, 256]
    wo = np.ascontiguousarray(woT.reshape(C2, 2, 128).transpose(1, 0, 2))
    ident = np.eye(128, dtype=np.float32)

    in_maps = []
    for core in range(NCORES):
        shard = np.ascontiguousarray(x[core * BPC:(core + 1) * BPC])
        in_maps.append({"x": shard, "wq": wq, "wg": wgq, "wo": wo,
                        "ident": ident, "identr": ident})
    return in_maps


def run(inputs, trace=False, **kw):
    nc = _get_nc()
    in_maps = prep_inputs(**inputs)
    res = run_bass_kernel_spmd(nc, in_maps, core_ids=list(range(NCORES)),
                               trace=trace, **kw)
    outs = [res.results[i]["out"] for i in range(NCORES)]
    full = np.concatenate(outs, axis=0).reshape(B, C, H, W).astype(np.float32)
    return full, res


def kernel(**inputs):
    full, _ = run(inputs, trace=False)
    return full
